# revision 39
# baseline (speedup 1.0000x reference)
"""ALiBi bidirectional attention — 8-core Trainium2 Bass kernel (v4).

Problem: B=2, T=2048, D=1024, H=16, hd=64, f32 in/out.
reference: softmax(Q K^T/8 + slopes_h * -|i-j|) V, then out-proj.

Sharding (sequence-parallel): core c handles batch c//4 and query rows
q0 = 512*(c%4) .. q0+512. Out-proj contracts the full model dim locally,
so the output is a pure concat of per-core [512, 1024] slices.

K is projected locally over a 1024-row HALO [q0-256, q0+768): the own
block plus the near tiles (local kt 4,5,14,15) that every head's kept
set touches. Only the far tiles (kt 6..13, kept by the 5 lightest heads
= 28 (kt,h) slices) come from the AllGather of the own K block (1MB->
4MB bf16 ring, ~60us) — and the heads needing them are processed LAST,
so the ring hides behind ~60us of local attention. The gather bounce is
DMA'd from the gpsimd queue: the collective trigger's wait semaphore is
a cumulative per-HW-queue DMA counter, and on the sync queue it would
alias the big input loads (measured +40us trigger delay). V is
recomputed in full locally (packed: only used (kt,h) slices).

SPMD rotation: k data lives in LOCAL coordinates (k_phys-q0) mod 2048;
gather-back DMAs take their source block from a host-passed table via
register-offset APs, so the graph is core-uniform.

ALiBi with s = bf16-snapped slope (q_lo in [0,512), kl = 128*kt + p):
  * crossing tiles (kt 0..3): scores exp'd raw then multiplied by the
    per-head window EW[p,col]=exp(-s|p-col+384|), generated on the fly.
  * rest tiles (kt 4..15): bias -s|diff| = per-partition exp bias
    (-+s*p + tile const, per (h,kt)) + per-query factor F shared by all
    non-wrap (exp(-s(511-q_lo))) resp. wrap (exp(-s*q_lo)) tiles; both
    factors <=1, so no overflow, and underflow only kills e^-80 terms.
    F multiplies the exp'd probs per tile (DVE), indexed by remote
    block so the graph stays core-uniform.
Scores are transposed (ST=[kpos,q]) so probs feed AV as lhsT; a ones
column in V yields softmax row-sums in the same matmul; no row-max pass
(args <= ~6). Rest tile kt kept iff s_h*min|diff|(kt) < 5 (74/192
survive; +2.5e-4 rel err vs the ~5e-3 bf16 floor). All scores matmuls
are contract-64 PAIRS in the PE's 64x128 row-tiled mode (even k-tiles
at SBUF partitions 0-63, odd at 64-127; Q^T duplicated to both halves;
the symmetric kept-sets always pair an even tile with an odd one).

Per-head normalize (z-extract/recip/broadcast/scale) is emitted with a
2-head lag so no engine queue blocks younger heads' early stages on an
older head's tail.
"""
import math
import sys

sys.path.insert(0, "/opt/trn_rl_repo")

import numpy as np

from concourse import bass, bacc
import concourse.tile as tile
from concourse.bass_utils import run_bass_kernel_spmd

mybir = bass.mybir
FP32 = mybir.dt.float32
BF16 = mybir.dt.bfloat16
INT32 = mybir.dt.int32

B, T, D = 2, 2048, 1024
H, HD = 16, 64
NCORES = 8
QS = 512                      # query rows per core
NKT = T // 128                # 16 k tiles
GROUPS = [[0, 1, 2, 3], [4, 5, 6, 7]]
NEAR = (4, 5, 14, 15)         # rest tiles inside the projected halo

try:
    import ml_dtypes
    BF16_NP = np.dtype(ml_dtypes.bfloat16)
except ImportError:
    BF16_NP = None


def _bf16_round_f32(x):
    u = np.asarray(x, np.float32).view(np.uint32)
    r = (u + 0x7FFF + ((u >> 16) & 1)) & 0xFFFF0000
    return r.astype(np.uint32).view(np.float32)


def _slopes():
    start = 2.0 ** (-(2.0 ** (-(math.log2(H) - 3))))
    return np.asarray([start * start ** i for i in range(H)], np.float32)


SLOPES = _bf16_round_f32(_slopes())     # used consistently everywhere

SKIP_THRESH = 5.0

# min |diff| over rest tile kt (local coords), kt = 4..15
_MINDIFF = [min(128 * kt - 511, 1921 - 128 * kt) for kt in range(4, 16)]

# kept rest tiles per head, single-tile granularity
KEPT = [[kt for kt in range(4, 16)
         if SLOPES[h] * _MINDIFF[kt - 4] < SKIP_THRESH] for h in range(H)]
FAR = [[kt for kt in KEPT[h] if kt not in NEAR] for h in range(H)]
# local-only heads first: the far tiles' AllGather hides behind them
HEAD_ORDER = sorted(range(H), key=lambda h: (len(FAR[h]), len(KEPT[h])))


def _rl(kt):
    return kt // 4


KEPT_RLS = [sorted({_rl(kt) for kt in KEPT[h]}) for h in range(H)]
FE_COL = {}
for _h in range(H):
    for _r in KEPT_RLS[_h]:
        FE_COL[(_h, _r)] = len(FE_COL)
NFE = len(FE_COL)

# score pairs per head: own (0,1),(2,3); near (4,5),(14,15) or (4,15);
# far (kt_even, 19-kt_even) — always one even (lower) + one odd (upper)
def _pairs(h):
    prs = []
    near = [kt for kt in NEAR if kt in KEPT[h]]
    if len(near) == 2:
        prs.append((4, 15))
    else:
        prs.extend([(4, 5), (14, 15)])
    for kte in (6, 8, 10, 12):
        if kte in FAR[h]:
            prs.append((kte, 19 - kte))
    return prs


# packed V storage: own tiles all heads; rest tiles only keeping heads
VOFF = {}
for _kt in range(4):
    for _h in range(H):
        VOFF[(_kt, _h)] = 16 * _kt + _h
_off = 64
for _kt in range(4, 16):
    for _h in range(H):
        if _kt in KEPT[_h]:
            VOFF[(_kt, _h)] = _off
            _off += 1
NV = _off

# V rest-tile chunk order: near tiles first (every head needs those)
_VORD = [4, 15, 5, 14, 6, 13, 7, 12, 8, 11, 9, 10]
V_CHUNKS = [(t, half) for t in _VORD for half in range(2)
            if any(t in KEPT[h] for h in range(8 * half, 8 * half + 8))]

# column of halo tile kt inside the paired kext2 [128, H, 512] layout
# (pair slots: (14,15), (0,1), (2,3), (4,5); even tile lower, odd upper)
_K2COL = {14: 0, 15: 0, 0: 128, 1: 128, 2: 256, 3: 256, 4: 384, 5: 384}

# --------------------------------------------------------------------------
# graph
# --------------------------------------------------------------------------


def _build_graph():
    nc = bacc.Bacc("TRN2", target_bir_lowering=False, debug=False,
                   num_devices=NCORES)

    p = {}
    p["xq"] = nc.declare_dram_parameter("xq", [D, QS], BF16, isOutput=False)
    p["xk"] = nc.declare_dram_parameter("xk", [D, 2 * QS], BF16, isOutput=False)
    p["xv"] = nc.declare_dram_parameter("xv", [D, T], BF16, isOutput=False)
    for nm in ("wq", "wk", "wv", "wo"):
        p[nm] = nc.declare_dram_parameter(nm, [D, D], BF16, isOutput=False)
    p["biasall"] = nc.declare_dram_parameter("biasall", [128, H * 12], FP32,
                                             isOutput=False)
    p["fet"] = nc.declare_dram_parameter("fet", [128, NFE * QS], BF16,
                                         isOutput=False)
    p["dbase"] = nc.declare_dram_parameter("dbase", [128, 896], FP32,
                                           isOutput=False)
    p["rotidx"] = nc.declare_dram_parameter("rotidx", [1, 4], INT32,
                                            isOutput=False)
    p["out"] = nc.declare_dram_parameter("out", [QS, D], FP32, isOutput=True)

    bounce_k = nc.dram_tensor("bounce_k", [D, QS], BF16)
    agk = nc.dram_tensor("agk", [4, D, QS], BF16)

    with tile.TileContext(nc) as tc:
        _emit(tc, nc, p, bounce_k, agk)

    nc.compile()
    return nc


def _emit(tc, nc, p, bounce_k, agk):
    Exp = mybir.ActivationFunctionType.Exp
    import contextlib
    ctx = contextlib.ExitStack()

    cpool = ctx.enter_context(tc.tile_pool(name="consts", bufs=1))
    kvq = ctx.enter_context(tc.tile_pool(name="kvq", bufs=1))
    xvp = ctx.enter_context(tc.tile_pool(name="xvp", bufs=1))
    wvp = ctx.enter_context(tc.tile_pool(name="wvp", bufs=1))

    rot_sb = cpool.tile([1, 4], INT32)
    nc.gpsimd.dma_start(rot_sb[:], p["rotidx"].ap())
    qt = kvq.tile([128, H, QS], BF16)           # Q^T, duplicated both halves
    kext2 = kvq.tile([128, H, 512], BF16)       # halo K^T pairs even|odd
    # packed V slices (+ones col), padded so AV lhsT can read a 128-col
    # window (full-width weights keep Fast Weight Load enabled)
    vfull = kvq.tile([128, NV * 65 + 63], BF16)
    vful3 = vfull[:, 0:NV * 65].rearrange("p (v c) -> p v c", c=65, v=NV)
    biasall = cpool.tile([128, H * 12], FP32)
    dbase = cpool.tile([128, 896], FP32)
    nc.gpsimd.dma_start(dbase[:], p["dbase"].ap())
    nc.gpsimd.dma_start(biasall[:], p["biasall"].ap())

    def cast(idx, dst, src):
        # alternate psum->sbuf casts across the two free engines
        if idx % 2 == 0:
            nc.scalar.copy(dst, src)
        else:
            nc.vector.tensor_copy(dst, src)

    # ================= phase 1: K-halo proj + gather, V own, Q ============
    pctx = contextlib.ExitStack()
    xw = pctx.enter_context(tc.tile_pool(name="xw", bufs=1))
    wrot = pctx.enter_context(tc.tile_pool(name="wrot", bufs=2))
    kl_pool = pctx.enter_context(tc.tile_pool(name="klp", bufs=1))

    def load_w(wt, wnm, eng):
        wsrc = p[wnm].ap().rearrange("(j p) c -> p j c", p=128)
        eng.dma_start(wt[:, 0, :], wsrc[:, 0, :])
        eng.dma_start(wt[:, 1:4, :], wsrc[:, 1:4, :])
        eng.dma_start(wt[:, 4:8, :], wsrc[:, 4:8, :])

    with tc.tile_pool(name="pp8", bufs=1, space="PSUM") as pp8:
        # ---- K halo projection: 1024 cols in two 512-col rounds ----------
        xk = xw.tile([128, 8, 2 * QS], BF16, tag="xk")
        wk_sb = wrot.tile([128, 8, D], BF16, tag="w")
        xsrc_k = p["xk"].ap().rearrange("(j p) c -> p j c", p=128)
        nc.sync.dma_start(xk[:, 0, :], xsrc_k[:, 0, :])
        load_w(wk_sb, "wk", nc.sync)
        nc.sync.dma_start(xk[:, 1:4, :], xsrc_k[:, 1:4, :])
        nc.sync.dma_start(xk[:, 4:8, :], xsrc_k[:, 4:8, :])
        # remaining inputs stream on the same queue in consumption order so
        # they never steal HBM bandwidth from an earlier stage's operands
        xv = xvp.tile([128, 8, T], BF16)
        wv_sb = wvp.tile([128, 8, D], BF16)
        xsrc_v = p["xv"].ap().rearrange("(j p) c -> p j c", p=128)
        nc.sync.dma_start(xv[:, :, 0:QS], xsrc_v[:, :, 0:QS])
        nc.sync.dma_start(wv_sb[:], p["wv"].ap().rearrange(
            "(j p) c -> p j c", p=128))

        klext = kl_pool.tile([64, H, 2 * QS], BF16)
        for rnd in range(2):
            psk = [pp8.tile([128, QS], FP32, tag=f"p{j}", name=f"psk{rnd}{j}")
                   for j in range(8)]
            for cj in range(8):
                for j in range(8):
                    nc.tensor.matmul(psk[j][:],
                                     wk_sb[:, cj, 128 * j:128 * (j + 1)],
                                     xk[:, cj, QS * rnd:QS * (rnd + 1)],
                                     start=(cj == 0), stop=(cj == 7))
            for j in range(8):
                cast(0, klext[:, 2 * j, QS * rnd:QS * (rnd + 1)],
                     psk[j][0:64, :])
                cast(1, klext[:, 2 * j + 1, QS * rnd:QS * (rnd + 1)],
                     psk[j][64:128, :])
        # bounce own block (ext cols 256:768) from the GPSIMD queue: the
        # collective trigger waits on this HW queue's cumulative DMA
        # counter, which must not alias the big sync/scalar input loads
        nc.gpsimd.dma_start(
            bounce_k.ap().rearrange("(h p) c -> p h c", p=64),
            klext[:, :, 256:768])
        nc.gpsimd.collective_compute(
            "AllGather", mybir.AluOpType.bypass, replica_groups=GROUPS,
            ins=[bounce_k.ap().opt()], outs=[agk.ap().opt()])
        # halo pair layout: even ext-tiles -> partitions 0-63, odd -> 64-127
        for sl in range(4):
            nc.sync.dma_start(kext2[0:64, :, 128 * sl:128 * (sl + 1)],
                              klext[:, :, 256 * sl:256 * sl + 128])
            nc.sync.dma_start(kext2[64:128, :, 128 * sl:128 * (sl + 1)],
                              klext[:, :, 256 * sl + 128:256 * sl + 256])
        # rotation registers early so the far-head K prefetches can issue
        # while the sync queue is otherwise idle
        rvs = []
        for rl in range(4):
            reg = nc.sync.alloc_register(f"rot{rl}")
            nc.sync.reg_load(reg, rot_sb[0:1, rl:rl + 1])
            rvs.append(nc.sync.snap(reg, donate=True))

        # ---- V projection (own 4 k-tiles), cj-outer over 8 psum banks ----
        nc.vector.memset(vful3[:, :, 64:65], 1.0)
        nc.vector.memset(vfull[:, NV * 65:], 0.0)
        psv = [pp8.tile([128, QS], FP32, tag=f"p{j}", name=f"psv{j}")
               for j in range(8)]
        for cj in range(8):
            for tc_i in range(4):
                for nh in range(2):
                    nc.tensor.matmul(psv[2 * tc_i + nh][:],
                                     xv[:, cj, 128 * tc_i:128 * (tc_i + 1)],
                                     wv_sb[:, cj, 512 * nh:512 * (nh + 1)],
                                     start=(cj == 0), stop=(cj == 7))
        for tc_i in range(4):
            for nh in range(2):
                cast(nh, vful3[:, 16 * tc_i + 8 * nh:16 * tc_i + 8 * nh + 8,
                               0:64],
                     psv[2 * tc_i + nh][:].rearrange("p (h d) -> p h d", h=8))

        # ---- Q projection (xq reuses xk's slot; rest of xv streams in) ---
        xq = xw.tile([128, 8, QS], BF16, tag="xq")
        wq_sb = wrot.tile([128, 8, D], BF16, tag="w")
        xsrc_q = p["xq"].ap().rearrange("(j p) c -> p j c", p=128)
        nc.scalar.dma_start(xq[:, 0:4, :], xsrc_q[:, 0:4, :])
        nc.scalar.dma_start(xq[:, 4:8, :], xsrc_q[:, 4:8, :])
        load_w(wq_sb, "wq", nc.scalar)
        nc.scalar.dma_start(xv[:, :, QS:T], xsrc_v[:, :, QS:T])
        psq = [pp8.tile([128, QS], FP32, tag=f"p{j}", name=f"psq{j}")
               for j in range(8)]
        for cj in range(8):
            for j in range(8):
                nc.tensor.matmul(psq[j][:], wq_sb[:, cj, 128 * j:128 * (j + 1)],
                                 xq[:, cj, :], start=(cj == 0), stop=(cj == 7))
        for j in range(8):
            # write both partition halves directly (row-tiled matmuls read
            # Q^T from partitions 0-63 and 64-127)
            cast(0, qt[0:64, 2 * j, :], psq[j][0:64, :])
            cast(1, qt[0:64, 2 * j + 1, :], psq[j][64:128, :])
            cast(1, qt[64:128, 2 * j, :], psq[j][0:64, :])
            cast(0, qt[64:128, 2 * j + 1, :], psq[j][64:128, :])

    agk_r = agk.ap().rearrange("r (h d) c -> r d h c", h=H)    # [4,64,H,QS]

    pctx.close()   # xw/wrot/klext SBUF freed for the attention pools

    # ================= phase 2: unified per-head attention ================
    late = ctx.enter_context(tc.tile_pool(name="late", bufs=1))
    fe = late.tile([128, NFE, QS], BF16)        # per-(h,rl) q-factors
    nc.gpsimd.dma_start(fe[:], p["fet"].ap().rearrange(
        "p (f c) -> p f c", f=NFE))
    wo_sb = late.tile([128, 8, D], BF16)
    nc.gpsimd.dma_start(wo_sb[:], p["wo"].ap().rearrange(
        "(j p) c -> p j c", p=128))
    with tc.tile_pool(name="otn", bufs=1) as otpool, \
         tc.tile_pool(name="ewp", bufs=3) as ewp, \
         tc.tile_pool(name="ktstream", bufs=5) as kts, \
         tc.tile_pool(name="exps", bufs=4) as epool, \
         tc.tile_pool(name="recip", bufs=2) as rpool, \
         tc.tile_pool(name="yout", bufs=1) as ypool, \
         tc.tile_pool(name="stps", bufs=2, space="PSUM") as stp, \
         tc.tile_pool(name="otps", bufs=3, space="PSUM") as otp, \
         tc.tile_pool(name="vrp", bufs=1, space="PSUM") as vrp:

        vi = 0

        def v_chunk():
            nonlocal vi
            if vi >= len(V_CHUNKS):
                return
            t, half = V_CHUNKS[vi]
            vi += 1
            hs = [h for h in range(8 * half, 8 * half + 8) if t in KEPT[h]]
            h0, n = hs[0], len(hs)
            c0 = 512 * half + 64 * (h0 - 8 * half)
            ps = vrp.tile([128, 512], FP32, tag="vr")
            for cj in range(8):
                nc.tensor.matmul(ps[:, 0:64 * n],
                                 xv[:, cj, 128 * t:128 * (t + 1)],
                                 wv_sb[:, cj, c0:c0 + 64 * n],
                                 start=(cj == 0), stop=(cj == 7))
            nc.vector.tensor_copy(
                vful3[:, VOFF[(t, h0)]:VOFF[(t, h0)] + n, 0:64],
                ps[:, 0:64 * n].rearrange("p (h d) -> p h d", h=n))

        ot = otpool.tile([128, 8, QS], BF16)    # normalized O^T

        # prefetch all far-head K tiles up front: the DMAs wait on the
        # AllGather internally while the sync queue has nothing else to do
        kth_of = {}
        for h in HEAD_ORDER:
            far_prs = [pr for pr in _pairs(h) if pr[0] not in NEAR]
            if not far_prs:
                continue
            kth = kts.tile([128, 4 * 128], BF16, tag="kth")
            kth_of[h] = kth
            for i, pr in enumerate(far_prs):
                for half, kt in enumerate(pr):
                    rl = _rl(kt)
                    c0 = 128 * (kt - 4 * rl)
                    nc.sync.dma_start(
                        kth[64 * half:64 * half + 64, 128 * i:128 * (i + 1)],
                        agk_r[bass.ds(rvs[rl], 1), :, h, c0:c0 + 128])

        pending = []                            # (h, otps) awaiting normalize

        def normalize(lag):
            while len(pending) > lag:
                h, otps = pending.pop(0)
                zrow = rpool.tile([1, QS], FP32, tag="zrow")
                nc.vector.tensor_copy(zrow[:], otps[64:65, :])
                rec = rpool.tile([1, QS], FP32, tag="rec")
                # approx recip needs partition-0 fp32; ~51 ULP is plenty
                nc.vector.reciprocal_approx_fast(rec[:], zrow[:])
                bcs = rpool.tile([64, QS], FP32, tag="bcs")
                nc.gpsimd.partition_broadcast(bcs[:], rec[:])
                nc.vector.tensor_mul(
                    ot[64 * (h % 2):64 * (h % 2) + 64, h // 2, :],
                    otps[0:64, :], bcs[:])

        for hi, h in enumerate(HEAD_ORDER):
            prs = _pairs(h)
            far_prs = [pr for pr in prs if pr[0] not in NEAR]
            kth = kth_of.get(h)
            # every V slice this head's AVs read must be projected first
            need = max(V_CHUNKS.index((t, h // 8)) for t in KEPT[h]
                       if (t, h // 8) in V_CHUNKS)
            while vi <= need:
                v_chunk()

            # on-the-fly EW window for the 4 crossing tiles of this head
            ew = ewp.tile([128, 896], BF16, tag="ew")
            nc.scalar.activation(ew[:], dbase[:], Exp, scale=-float(SLOPES[h]))

            # ---- scores: own pairs then near/far pairs, all row-tiled ----
            sts = []
            for g in range(2):          # own (0,1) and (2,3) from the halo
                stps = stp.tile([128, 2 * QS], FP32, tag="st")
                c0 = _K2COL[2 * g]
                nc.tensor.matmul(stps[:, 0:QS],
                                 kext2[0:64, h, c0:c0 + 128],
                                 qt[0:64, h, :], start=True, stop=True)
                nc.tensor.matmul(stps[:, QS:2 * QS],
                                 kext2[64:128, h, c0:c0 + 128],
                                 qt[64:128, h, :], start=True, stop=True)
                sts.append(stps)

            otps = otp.tile([128, QS], FP32, tag="ot")
            nmm = 4 + len(KEPT[h])
            mi = 0

            def av(kt, e_half):
                nonlocal mi
                off = 65 * VOFF[(kt, h)]
                nc.tensor.matmul(otps[:], vfull[:, off:off + 128], e_half,
                                 start=(mi == 0), stop=(mi == nmm - 1))
                mi += 1

            # own-block: exp raw scores, EW-window multiply, AV kt 0..3
            for g in range(2):
                e = epool.tile([128, 2 * QS], BF16, tag="e")
                nc.scalar.activation(e[:], sts[g][:], Exp)
                for j in range(2):
                    kt = 2 * g + j
                    nc.vector.tensor_mul(e[:, QS * j:QS * (j + 1)],
                                         e[:, QS * j:QS * (j + 1)],
                                         ew[:, 384 - 128 * kt:896 - 128 * kt])
                    av(kt, e[:, QS * j:QS * (j + 1)])
                v_chunk()

            # rest pairs: contract-64 scores -> exp -> F-mult (DVE) -> AV
            for i, pr in enumerate(prs):
                is_far = pr[0] not in NEAR
                stps = stp.tile([128, 2 * QS], FP32, tag="st")
                for half, kt in enumerate(pr):
                    if is_far:
                        lhsT = kth[64 * half:64 * half + 64,
                                   128 * far_prs.index(pr):
                                   128 * far_prs.index(pr) + 128]
                    else:
                        c0 = _K2COL[kt]
                        lhsT = kext2[64 * half:64 * half + 64, h,
                                     c0:c0 + 128]
                    nc.tensor.matmul(stps[:, QS * half:QS * (half + 1)], lhsT,
                                     qt[64 * half:64 * half + 64, h, :],
                                     start=True, stop=True)
                e = epool.tile([128, 2 * QS], BF16, tag="e")
                for half, kt in enumerate(pr):
                    eh = e[:, QS * half:QS * (half + 1)]
                    nc.scalar.activation(
                        eh, stps[:, QS * half:QS * (half + 1)], Exp,
                        bias=biasall[:, 12 * h + kt - 4:12 * h + kt - 3])
                    nc.vector.tensor_mul(
                        eh, eh, fe[:, FE_COL[(h, _rl(kt))], :])
                    av(kt, eh)
                v_chunk()

            pending.append((h, otps))
            normalize(1)    # 1-head lag: no queue blocks on older tails

        while vi < len(V_CHUNKS):
            v_chunk()
        normalize(0)

        # --- out-projection (reuses the "ot" psum slots) ---
        # accumulate head-pair columns in readiness order so the chain ends
        # on the pair whose heads were normalized last
        jord = sorted(range(8), key=lambda j: max(
            HEAD_ORDER.index(2 * j), HEAD_ORDER.index(2 * j + 1)))
        for tc_i in range(4):
            y = ypool.tile([128, D], FP32, tag="y")
            for nh in range(2):
                ps = otp.tile([128, 512], FP32, tag="ot", name=f"ops{tc_i}{nh}")
                for ji, j in enumerate(jord):
                    nc.tensor.matmul(ps[:], ot[:, j, 128 * tc_i:128 * (tc_i + 1)],
                                     wo_sb[:, j, 512 * nh:512 * (nh + 1)],
                                     start=(ji == 0), stop=(ji == 7))
                cast(nh, y[:, 512 * nh:512 * (nh + 1)], ps[:])
            nc.sync.dma_start(p["out"].ap()[128 * tc_i:128 * (tc_i + 1), :], y[:])

    ctx.close()


# --------------------------------------------------------------------------
# host side
# --------------------------------------------------------------------------


def _prep_core_inputs(inputs, c):
    b, s = divmod(c, 4)
    q0 = QS * s
    sl = slice(q0, q0 + QS)
    f32 = np.float32

    for bn in ("bq", "bk", "bv", "bo"):
        assert not np.any(np.asarray(inputs[bn])), \
            f"nonzero {bn} not supported by this kernel build"

    def tr(x):
        return np.ascontiguousarray(np.asarray(x, f32).T)

    xv_rot = np.roll(tr(inputs["value"][b]), -q0, axis=1)  # local coords
    # K halo: phys rows [q0-256, q0+768) = ext-coordinate columns 0..1024
    xk_ext = np.roll(tr(inputs["key"][b]), -(q0 - 256), axis=1)[:, 0:2 * QS]
    m = {
        "xq": tr(inputs["query"][b][sl]).astype(BF16_NP),
        "xk": np.ascontiguousarray(xk_ext).astype(BF16_NP),
        "xv": xv_rot.astype(BF16_NP),
        "wq": (np.asarray(inputs["Wq"], f32) * HD ** -0.5).astype(BF16_NP),
        "wk": np.asarray(inputs["Wk"], f32).astype(BF16_NP),
        "wv": np.asarray(inputs["Wv"], f32).astype(BF16_NP),
        "wo": np.asarray(inputs["Wo"], f32).astype(BF16_NP),
    }

    # wrap iff local kl >= T - q0 (block-aligned, so per rest tile/block)
    pvec = np.arange(128, dtype=f32)
    qvec = np.arange(QS, dtype=f32)
    biasall = np.zeros((128, H, 12), f32)
    fet = np.zeros((128, NFE, QS), f32)
    for h in range(H):
        sh = SLOPES[h]
        for kt in range(4, 16):
            wrap = (128 * kt) >= (T - q0) if q0 > 0 else False
            if wrap:
                biasall[:, h, kt - 4] = sh * pvec - sh * (T - 128 * kt)
            else:
                biasall[:, h, kt - 4] = -sh * (pvec + 128 * kt - 511)
        for rl in KEPT_RLS[h]:
            wrap = (512 * rl) >= (T - q0) if q0 > 0 else False
            f = np.exp(-sh * qvec) if wrap else np.exp(-sh * (511.0 - qvec))
            fet[:, FE_COL[(h, rl)], :] = f[None, :]
    m["biasall"] = biasall.reshape(128, H * 12)
    m["fet"] = fet.reshape(128, NFE * QS).astype(BF16_NP)

    col = np.arange(896, dtype=f32)
    m["dbase"] = np.abs(pvec[:, None] - col[None, :] + 384.0).astype(f32)

    m["rotidx"] = np.asarray([[(rl + s) % 4 for rl in range(4)]], np.int32)
    return m


_NC_CACHE = {}


def _get_nc():
    if "nc" not in _NC_CACHE:
        _NC_CACHE["nc"] = _build_graph()
    return _NC_CACHE["nc"]


def run(inputs, trace=False, trace_kwargs=None):
    nc = _get_nc()
    in_maps = [_prep_core_inputs(inputs, c) for c in range(NCORES)]
    res = run_bass_kernel_spmd(nc, in_maps, list(range(NCORES)),
                               trace=trace, **(trace_kwargs or {}))
    out = np.empty((B, T, D), np.float32)
    for c in range(NCORES):
        b, s = divmod(c, 4)
        out[b, QS * s:QS * (s + 1), :] = res.results[c]["out"]
    return out, res


def kernel(**inputs):
    return run(inputs)[0]
